# revision 1
# baseline (speedup 1.0000x reference)
"""Trainium2 Bass kernel for nn_EdgeEncoder (moe_routing).

Strategy
--------
Each of E edges is routed to 1 of 9 expert MLPs (4 -> 256 -> 256), then
  out = relu(concat([type_embed[tid], source_embed[sid], pv]) @ Wf + bf).

Host (numpy, cheap O(E) work):
  * scale/mask params, group edge indices by expert (base type),
  * split every expert's edges evenly over the 8 cores, padding each
    per-core expert segment to a multiple of 128 edges so all cores run
    ONE identical program (segment boundaries are compile-time constants),
  * algebraic fusions so the device does minimal work:
      - x gets a ones-row so b1 rides inside the layer-1 matmul,
      - V[t] = W2[t] @ Wf_pv (f64 host precompute) fuses layer 2 with the
        final projection: pv @ Wf_pv == h @ V[t] + const,
      - G_t = [type_embed @ Wf_t ; source_embed @ Wf_s ; b2@Wf_pv + bf]
        turns both embedding gathers and every bias into one K=20 matmul
        against the one-hot rows (ones-row coefficient carries the consts).

Device per 512-edge block (edges pre-grouped by expert, transposed):
  hT  = relu(W1e[t].T @ xT1)            2 matmuls K=5   N=512   (PSUM 2 banks)
  outT= G_t.T-rows @ uT + V[t].T-chunks @ hT    6 matmuls K=20/128 N=512 (2 banks)
  relu PSUM->SBUF, DMA outT tiles to DRAM [D, L]; host un-permutes.

Matmuls run as float32r: 1 cycle/row. A short bf16 warm-up burst raises the
PE clock gate (HAM) at kernel start; the fp32-HIGH stream itself is
discounted by the HAM and would otherwise run at 1.2 GHz throughout.
"""

import math
import os

import ml_dtypes
import numpy as np

import concourse.bacc as bacc
import concourse.bass as bass
import concourse.mybir as mybir
import concourse.tile as tile
from concourse.bass_utils import run_bass_kernel_spmd

# ---- static module configuration (mirrors the torch source) ----
T = 9            # base types ("experts")
P_MAX = 4
D = 256
N_TYPES = 14
N_SRC = 5
NCORES = 8
BLOCK = 512      # edges per device block (one PSUM bank of fp32)
GRP = 128        # edge group granularity (PE partition dim)

BASE_MAP = np.array([0, 0, 0, 1, 1, 1, 2, 2, 3, 4, 5, 6, 7, 8], dtype=np.int32)
PCOUNT = np.array([2, 2, 1, 1, 1, 1, 3, 2, 4], dtype=np.int32)
SCALES = np.ones((T, P_MAX), dtype=np.float32)
SCALES[0, :2] = [1.0, 1e-06]      # nmos  m, w
SCALES[1, :2] = [1.0, 1e-06]      # pmos  m, w
SCALES[2, 0] = 1.0                # balun rout
SCALES[3, 0] = 1000.0             # resistor r
SCALES[4, 0] = 1e-12              # capacitor c
SCALES[5, 0] = 1e-09              # inductor l
SCALES[6, :3] = [1.0, 1.0, 1.0]   # vsource dc, mag, phase
SCALES[7, :2] = [0.001, 0.001]    # isource dc, mag
SCALES[8, :4] = [1.0, 1.0, 1e9, 1.0]  # port dbm, dc, freq, num

KX = 5                            # x rows: xT(4) + ones
KU = N_TYPES + N_SRC + 1          # 20 rows: type/source one-hot + ones

_MM_DT = (mybir.dt.float32 if os.environ.get("EDGEENC_MM_DT") == "float32"
          else mybir.dt.float32r)
_F32 = mybir.dt.float32
_BF16 = mybir.dt.bfloat16
# G matmul as bf16 hi+lo pair (2x rows, full HAM credit) vs one f32r pass
_G_SPLIT = os.environ.get("EDGEENC_G_SPLIT", "0") == "1"
# dense bf16 warm-up burst: the PE HAM clock gate never un-throttles on the
# kernel's own fp32-HIGH stream, so warm it explicitly at the start
_WARM_BURST = int(os.environ.get("EDGEENC_WARM_BURST", "24"))

_PROGRAM_CACHE: dict = {}
LAST_RESULT = None  # BassKernelResults of the most recent run (for test harness)


def _layout(base_ids: np.ndarray):
    """Per-expert per-core segment sizes (multiples of GRP), identical on
    every core so one program serves all 8."""
    n_t = np.bincount(base_ids, minlength=T)
    m_t = np.zeros(T, dtype=np.int64)
    for t in range(T):
        if n_t[t] > 0:
            per_core = math.ceil(n_t[t] / NCORES)
            m_t[t] = math.ceil(per_core / GRP) * GRP
    L0 = int(m_t.sum())
    L = math.ceil(L0 / BLOCK) * BLOCK
    # fold the tail pad into the last present expert's segment
    last = int(np.nonzero(m_t)[0][-1])
    m_t[last] += L - L0
    return n_t, m_t, L


def _group_experts(m_t: np.ndarray) -> np.ndarray:
    """expert id of each 128-edge group, concatenated per expert."""
    return np.repeat(np.arange(T), (m_t // GRP))


def _build_order(base_ids: np.ndarray, n_t, m_t, L) -> np.ndarray:
    """ORD[c, j] = global edge index at per-core slot j (or -1 = pad)."""
    ORD = np.full((NCORES, L), -1, dtype=np.int64)
    off = 0
    for t in range(T):
        if m_t[t] == 0:
            continue
        seg = int(m_t[t])
        idx = np.nonzero(base_ids == t)[0]
        arr = np.full(NCORES * seg, -1, dtype=np.int64)
        arr[: idx.shape[0]] = idx
        ORD[:, off : off + seg] = arr.reshape(NCORES, seg)
        off += seg
    return ORD


def _host_inputs(type_ids, source_ids, params, ORD):
    """INX[c] = [5, L]: xT (scaled/masked) + ones row.
    INU[c] = [20, L]: type one-hot, source one-hot, ones row."""
    base_ids = BASE_MAP[type_ids]
    scales = SCALES[base_ids]                                  # [E,4]
    validp = np.arange(P_MAX)[None, :] < PCOUNT[base_ids][:, None]
    x = np.where(validp, params.astype(np.float32) / scales, 0.0).astype(np.float32)

    L = ORD.shape[1]
    INX = np.zeros((NCORES, KX, L), dtype=np.float32)
    INU = np.zeros((NCORES, KU, L), dtype=np.float32)
    valid = ORD >= 0
    ids = ORD[valid]
    tmp = np.zeros((NCORES, L, P_MAX), dtype=np.float32)
    tmp[valid] = x[ids]
    INX[:, 0:P_MAX, :] = tmp.transpose(0, 2, 1)
    INX[:, P_MAX, :] = valid
    ci, co = np.nonzero(valid)
    INU[ci, type_ids[ids], co] = 1.0
    INU[ci, N_TYPES + source_ids[ids], co] = 1.0
    INU[:, KU - 1, :] = valid
    return INX, INU


def _host_weights(type_embed, source_embed, W1, b1, W2, b2, Wf, bf):
    f = np.float32
    W1 = W1.astype(f); b1 = b1.astype(f); W2 = W2.astype(np.float64)
    b2 = b2.astype(f); Wf = Wf.astype(f); bf = bf.astype(f)
    type_embed = type_embed.astype(f); source_embed = source_embed.astype(f)

    # layer1 lhsT blocks: [5, 9*256]; block t at cols [t*256,(t+1)*256)
    W1e = np.concatenate([W1, b1[:, None, :]], axis=1)          # [9,5,256]
    W1E = np.ascontiguousarray(W1e.transpose(1, 0, 2).reshape(KX, T * D))

    Wft, Wfs, Wfp = Wf[:D], Wf[D : 2 * D], Wf[2 * D :]

    # V[t] = W2[t] @ Wf_pv (f64), fusing layer 2 with the final projection.
    # lhsT blocks: [128, 18*256]; block (t,h) = V[t][h*128:(h+1)*128,:]
    V = (W2 @ Wfp.astype(np.float64)).astype(f)                 # [9,256,256]
    VR = np.ascontiguousarray(
        V.reshape(T, 2, 128, D).transpose(2, 0, 1, 3).reshape(128, T * 2 * D)
    )

    # G_t [20,256]: type rows, source rows, const row (b2@Wf_pv + bf)
    gt = type_embed @ Wft                                       # [14,256]
    gs = source_embed @ Wfs                                     # [5,256]
    gc = b2 @ Wfp + bf[None, :]                                 # [9,256]
    G = np.stack([np.concatenate([gt, gs, gc[t : t + 1]], axis=0) for t in range(T)])
    GSB = np.ascontiguousarray(G.transpose(1, 0, 2).reshape(KU, T * D))
    # optional bf16 hi+lo split (u is one-hot, so this is ~fp32-accurate)
    GHI = GSB.astype(ml_dtypes.bfloat16)
    GLO = (GSB - GHI.astype(f)).astype(ml_dtypes.bfloat16)
    return W1E, VR, GSB, GHI, GLO


def _build_program(m_t: tuple, L: int):
    """One compiled SPMD program for the given segment layout."""
    key = (m_t, L, str(_MM_DT), _G_SPLIT, _WARM_BURST)
    if key in _PROGRAM_CACHE:
        return _PROGRAM_CACHE[key]

    group_expert = _group_experts(np.asarray(m_t, dtype=np.int64))
    NB = L // BLOCK
    GP = BLOCK // GRP  # groups per block = 4

    nc = bacc.Bacc("TRN2", target_bir_lowering=False, debug=False,
                   num_devices=NCORES)
    inx_d = nc.dram_tensor("inx", [KX, L], _MM_DT, kind="ExternalInput")
    u_dt = _BF16 if _G_SPLIT else _MM_DT
    inu_d = nc.dram_tensor("inu", [KU, L], u_dt, kind="ExternalInput")
    w1e_d = nc.dram_tensor("w1e", [KX, T * D], _MM_DT, kind="ExternalInput")
    vr_d = nc.dram_tensor("vr", [128, T * 2 * D], _MM_DT, kind="ExternalInput")
    if _G_SPLIT:
        ghi_d = nc.dram_tensor("ghi", [KU, T * D], _BF16, kind="ExternalInput")
        glo_d = nc.dram_tensor("glo", [KU, T * D], _BF16, kind="ExternalInput")
    else:
        g_d = nc.dram_tensor("gsb", [KU, T * D], _MM_DT, kind="ExternalInput")
    out_d = nc.dram_tensor("out", [D, L], _F32, kind="ExternalOutput")

    RELU = mybir.ActivationFunctionType.Relu

    with tile.TileContext(nc) as tc:
        with (
            tc.tile_pool(name="wts", bufs=1) as wts,
            tc.tile_pool(name="inp", bufs=1) as inp,
            tc.tile_pool(name="hsb", bufs=6) as hsbp,
            tc.tile_pool(name="osb", bufs=6) as osbp,
            tc.tile_pool(name="hps", bufs=4, space=bass.MemorySpace.PSUM) as hps,
            tc.tile_pool(name="ops", bufs=4, space=bass.MemorySpace.PSUM) as ops,
        ):
            w1e = wts.tile([128, T * D], _MM_DT)
            vr = wts.tile([128, T * 2 * D], _MM_DT)
            # V (2.25MB) goes on the sync queue, which is otherwise idle
            # until the first output stores ~10us in
            nc.sync.dma_start(vr[:], vr_d.ap())
            nc.vector.memset(w1e[:].bitcast(_F32), 0.0)
            nc.gpsimd.dma_start(w1e[0:KX, :], w1e_d.ap())
            if _G_SPLIT:
                ghi = wts.tile([KU, T * D], _BF16)
                glo = wts.tile([KU, T * D], _BF16)
                nc.gpsimd.dma_start(ghi[:], ghi_d.ap())
                nc.gpsimd.dma_start(glo[:], glo_d.ap())
                gmats = (ghi, glo)
            else:
                gsb = wts.tile([128, T * D], _MM_DT)
                nc.vector.memset(gsb[:].bitcast(_F32), 0.0)
                nc.gpsimd.dma_start(gsb[0:KU, :], g_d.ap())
                gmats = (gsb,)

            # bf16 scratch operands for the HAM warm-up burst
            if _WARM_BURST:
                wmw = wts.tile([128, 128], _BF16)
                wma = wts.tile([128, BLOCK], _BF16)
                nc.vector.memset(wmw[:], 0.0)
                nc.vector.memset(wma[:], 0.0)
                wmp = hps.tile([GRP, BLOCK], _F32, name="warmps", tag="hts")
                for i in range(_WARM_BURST):
                    nc.tensor.matmul(wmp[:], wmw[:], wma[:], start=True,
                                     stop=True)

            # persistent input buffers, zero-padded to K=128 partitions so
            # every matmul runs full-row (HAM activity counts whole rows)
            NIB = min(6, NB)
            xts = [inp.tile([128, BLOCK], _MM_DT, name=f"xtile{j}", tag=f"xtile{j}")
                   for j in range(NIB)]
            uts = [inp.tile([128, BLOCK], u_dt, name=f"utile{j}", tag=f"utile{j}")
                   for j in range(NIB)]
            for j in range(NIB):
                # memset rejects float32r at ISA level; bitcast to f32
                nc.vector.memset(xts[j][:].bitcast(_F32), 0.0)
                nc.vector.memset(uts[j][:].bitcast(_F32), 0.0)

            # prefetch the first blocks' inputs ahead of the 2.25MB V DMA
            for b in range(min(2, NIB)):
                nc.gpsimd.dma_start(
                    xts[b][0:KX, :], inx_d.ap()[:, b * BLOCK : (b + 1) * BLOCK])
                nc.gpsimd.dma_start(
                    uts[b][0:KU, :], inu_d.ap()[:, b * BLOCK : (b + 1) * BLOCK])

            for b in range(NB):
                g0 = b * GP
                experts = [int(group_expert[g0 + i]) for i in range(GP)]
                # runs of equal expert: (t, col0, col1) relative to block
                runs = []
                for i, t in enumerate(experts):
                    if runs and runs[-1][0] == t:
                        runs[-1] = (t, runs[-1][1], (i + 1) * GRP)
                    else:
                        runs.append((t, i * GRP, (i + 1) * GRP))

                xt_t = xts[b % NIB]
                ut_t = uts[b % NIB]
                if b >= 2:
                    nc.gpsimd.dma_start(
                        xt_t[0:KX, :], inx_d.ap()[:, b * BLOCK : (b + 1) * BLOCK])
                    nc.gpsimd.dma_start(
                        ut_t[0:KU, :], inu_d.ap()[:, b * BLOCK : (b + 1) * BLOCK])

                # ---- layer 1: hT[h] = relu(W1e[t].T @ xT1) ----
                hts = [hps.tile([GRP, BLOCK], _F32, name=f"hts{b}_{j}", tag="hts")
                       for j in range(2)]
                for (t, c0, c1) in runs:
                    for h in range(2):
                        nc.tensor.matmul(
                            hts[h][:, c0:c1],
                            w1e[:, t * D + h * GRP : t * D + (h + 1) * GRP],
                            xt_t[:, c0:c1],
                            start=True, stop=True,
                        )
                hsb = [hsbp.tile([GRP, BLOCK], _MM_DT, name=f"hsb{b}_{j}", tag="hsb")
                       for j in range(2)]
                nc.scalar.activation(hsb[0][:], hts[0][:], RELU)
                nc.scalar.activation(hsb[1][:], hts[1][:], RELU)

                # ---- fused final: outT[n,e] = G_t.T @ uT + V[t].T-chunks @ hT ----
                ots = [ops.tile([GRP, BLOCK], _F32, name=f"ots{b}_{j}", tag="ots")
                       for j in range(2)]
                for (t, c0, c1) in runs:
                    # one accumulation group per expert run per bank; MMs
                    # alternate between the two n-half banks so consecutive
                    # instructions never target the same PSUM bank
                    for g in range(2):
                        nc.tensor.matmul(
                            ots[g][:, c0:c1],
                            vr[:, (t * 2 + 0) * D + g * GRP
                               : (t * 2 + 0) * D + (g + 1) * GRP],
                            hsb[0][:, c0:c1],
                            start=True, stop=False,
                        )
                    for gmat in gmats:
                        for g in range(2):
                            nc.tensor.matmul(
                                ots[g][:, c0:c1],
                                gmat[:, t * D + g * GRP : t * D + (g + 1) * GRP],
                                ut_t[:, c0:c1],
                                start=False, stop=False,
                            )
                    for g in range(2):
                        nc.tensor.matmul(
                            ots[g][:, c0:c1],
                            vr[:, (t * 2 + 1) * D + g * GRP
                               : (t * 2 + 1) * D + (g + 1) * GRP],
                            hsb[1][:, c0:c1],
                            start=False, stop=True,
                        )
                osb = [osbp.tile([GRP, BLOCK], _F32, name=f"osb{b}_{j}", tag="osb")
                       for j in range(2)]
                nc.vector.tensor_scalar_max(osb[0][:], ots[0][:], 0.0)
                nc.vector.tensor_scalar_max(osb[1][:], ots[1][:], 0.0)
                for g in range(2):
                    nc.sync.dma_start(
                        out_d.ap()[g * GRP : (g + 1) * GRP,
                                   b * BLOCK : (b + 1) * BLOCK],
                        osb[g][:],
                    )

    nc.compile()
    _PROGRAM_CACHE[key] = nc
    return nc


def kernel(type_ids, source_ids, params, type_embed, source_embed,
           W1, b1, W2, b2, Wf, bf):
    global LAST_RESULT
    type_ids = np.asarray(type_ids, dtype=np.int32)
    source_ids = np.asarray(source_ids, dtype=np.int32)
    params = np.asarray(params, dtype=np.float32)
    E = type_ids.shape[0]

    base_ids = BASE_MAP[type_ids]
    n_t, m_t, L = _layout(base_ids)
    ORD = _build_order(base_ids, n_t, m_t, L)
    INX, INU = _host_inputs(type_ids, source_ids, params, ORD)
    W1E, VR, GSB, GHI, GLO = _host_weights(
        np.asarray(type_embed), np.asarray(source_embed),
        np.asarray(W1), np.asarray(b1), np.asarray(W2), np.asarray(b2),
        np.asarray(Wf), np.asarray(bf))

    nc = _build_program(tuple(int(v) for v in m_t), L)

    in_maps = []
    for c in range(NCORES):
        m = {"inx": np.ascontiguousarray(INX[c]), "w1e": W1E, "vr": VR}
        if _G_SPLIT:
            m["inu"] = np.ascontiguousarray(INU[c].astype(ml_dtypes.bfloat16))
            m["ghi"] = GHI
            m["glo"] = GLO
        else:
            m["inu"] = np.ascontiguousarray(INU[c])
            m["gsb"] = GSB
        in_maps.append(m)

    trace = bool(int(os.environ.get("EDGEENC_TRACE", "0")))
    res = run_bass_kernel_spmd(nc, in_maps, core_ids=list(range(NCORES)),
                               trace=trace)
    LAST_RESULT = res

    full = np.zeros((E, D), dtype=np.float32)
    for c in range(NCORES):
        sel = ORD[c] >= 0
        oc = res.results[c]["out"]                     # [D, L]
        full[ORD[c][sel]] = np.ascontiguousarray(oc[:, sel].T)
    return full



# revision 6
# speedup vs baseline: 1.1659x; 1.1659x over previous
"""Trainium2 Bass kernel for nn_EdgeEncoder (moe_routing).

Strategy
--------
Each of E edges is routed to 1 of 9 expert MLPs (4 -> 256 -> 256), then
  out = relu(concat([type_embed[tid], source_embed[sid], pv]) @ Wf + bf).

Host (numpy, cheap O(E) work):
  * scale/mask params; group edges by expert (base type), and within an
    expert by (type_id, source_id) pair; DEAL each pair's edges round-robin
    across the 8 cores so every core sees identical pair-run boundaries
    (ceil(n_pair/8) columns each) -> one compiled program serves all cores,
  * algebraic fusions so the device does minimal work:
      - x gets a ones-row so b1 rides inside the layer-1 matmul,
      - V[t] = W2[t] @ Wf_pv (f64 host precompute) fuses layer 2 with the
        final projection: pv @ Wf_pv == h @ V[t] + const,
      - the whole "embedding" term  C[tid,sid] = type_embed[tid] @ Wf_t
        + source_embed[sid] @ Wf_s + b2[t] @ Wf_pv + bf  is constant per
        (tid,sid) pair.  Because edges are sorted by pair, C is piecewise
        constant along the edge axis and rides FOR FREE as the per-partition
        bias operand of the PSUM->SBUF relu evacuation ops.  This deletes
        the K=20 one-hot matmuls entirely (8 -> 6 PE passes per block).

Device per 512-edge block (edges pre-grouped, transposed):
  hT  = relu(W1e[t].T @ xT1)           2 matmuls fp32r K=5(pad 128) N=512
  outT[g] += V[t]-chunk(h).T @ hT[h]   4 matmuls bf16  K=128        N=512
  relu(outT + C[pair]) PSUM -> bf16 SBUF (bias-relu ops split between the
  Scalar and Vector engines, one slice per pair-run), DMA outT tiles to
  DRAM [D, L] bf16; host un-permutes and upcasts.

All matmuls are full 128-row (inputs zero-padded): partial-K matmuls run
in PE tile mode and starve the HAM activity monitor, dropping the PE to
1.2 GHz.  A short bf16 warm-up burst raises the HAM clock gate at kernel
start while the first DMAs are in flight.
"""

import math
import os

import ml_dtypes
import numpy as np

import concourse.bacc as bacc
import concourse.bass as bass
import concourse.mybir as mybir
import concourse.tile as tile
from concourse.bass_utils import run_bass_kernel_spmd

# ---- static module configuration (mirrors the torch source) ----
T = 9            # base types ("experts")
P_MAX = 4
D = 256
N_TYPES = 14
N_SRC = 5
NCORES = 8
BLOCK = 512      # edges per device block (one PSUM bank of fp32)
GRP = 128        # expert segment granularity (PE partition dim)

BASE_MAP = np.array([0, 0, 0, 1, 1, 1, 2, 2, 3, 4, 5, 6, 7, 8], dtype=np.int32)
PCOUNT = np.array([2, 2, 1, 1, 1, 1, 3, 2, 4], dtype=np.int32)
SCALES = np.ones((T, P_MAX), dtype=np.float32)
SCALES[0, :2] = [1.0, 1e-06]      # nmos  m, w
SCALES[1, :2] = [1.0, 1e-06]      # pmos  m, w
SCALES[2, 0] = 1.0                # balun rout
SCALES[3, 0] = 1000.0             # resistor r
SCALES[4, 0] = 1e-12              # capacitor c
SCALES[5, 0] = 1e-09              # inductor l
SCALES[6, :3] = [1.0, 1.0, 1.0]   # vsource dc, mag, phase
SCALES[7, :2] = [0.001, 0.001]    # isource dc, mag
SCALES[8, :4] = [1.0, 1.0, 1e9, 1.0]  # port dbm, dc, freq, num

KX = 5                            # x rows: xT(4) + ones
_F32 = mybir.dt.float32
_F32R = mybir.dt.float32r
_BF16 = mybir.dt.bfloat16
_WARM_BURST = int(os.environ.get("EDGEENC_WARM_BURST", "16"))

_PROGRAM_CACHE: dict = {}
LAST_RESULT = None  # BassKernelResults of the most recent run (for test harness)


def _layout(type_ids, source_ids):
    """Sort edges by (expert, type, source); deal each (t,tid,sid) group
    round-robin over cores.  Returns:
      ORD    [NCORES, L] global edge index per slot (-1 = pad)
      m_t    [T] per-core expert segment sizes (multiples of GRP)
      ranges list of (c0, c1, pair_col) bias runs, identical on all cores
      pairs  list of (t, tid, sid) in pair_col order (col 0 = zero bias)
      L      padded per-core column count (multiple of BLOCK)
    """
    base_ids = BASE_MAP[type_ids]
    m_t = np.zeros(T, dtype=np.int64)
    cols = []          # per expert: list of (k_p, pair_id, idx array)
    pairs = [None]     # pair_col 0 reserved for zero bias (pad columns)
    ranges = []
    ORD_parts = []
    off = 0
    for t in range(T):
        sel = np.nonzero(base_ids == t)[0]
        if sel.shape[0] == 0:
            continue
        key = type_ids[sel].astype(np.int64) * N_SRC + source_ids[sel]
        order = np.argsort(key, kind="stable")
        sel = sel[order]
        key = key[order]
        uk, counts = np.unique(key, return_counts=True)
        seg = 0
        pos = 0
        ord_t = []
        for u, n in zip(uk, counts):
            k_p = math.ceil(n / NCORES)
            pcol = len(pairs)
            pairs.append((t, int(u) // N_SRC, int(u) % N_SRC))
            idx = np.full(NCORES * k_p, -1, dtype=np.int64)
            idx[:n] = sel[pos : pos + n]
            pos += n
            # slot j of core c <- idx[j*NCORES + c]
            ord_t.append(idx.reshape(k_p, NCORES).T)     # [NCORES, k_p]
            ranges.append((off + seg, off + seg + k_p, pcol))
            seg += k_p
        m = math.ceil(seg / GRP) * GRP
        if m > seg:
            ranges.append((off + seg, off + m, 0))
            ord_t.append(np.full((NCORES, m - seg), -1, dtype=np.int64))
        m_t[t] = m
        ORD_parts.append(np.concatenate(ord_t, axis=1))
        off += m
    L0 = off
    L = math.ceil(L0 / BLOCK) * BLOCK
    if L > L0:
        ranges.append((L0, L, 0))
        ORD_parts.append(np.full((NCORES, L - L0), -1, dtype=np.int64))
        m_t[np.nonzero(m_t)[0][-1]] += L - L0
    ORD = np.concatenate(ORD_parts, axis=1)
    return ORD, m_t, ranges, pairs, L


def _host_inputs(type_ids, source_ids, params, ORD):
    """INX[c] = [5, L] fp32: xT (scaled/masked) + ones row."""
    base_ids = BASE_MAP[type_ids]
    scales = SCALES[base_ids]                                  # [E,4]
    validp = np.arange(P_MAX)[None, :] < PCOUNT[base_ids][:, None]
    x = np.where(validp, params.astype(np.float32) / scales, 0.0).astype(np.float32)

    L = ORD.shape[1]
    INX = np.zeros((NCORES, KX, L), dtype=np.float32)
    valid = ORD >= 0
    ids = ORD[valid]
    tmp = np.zeros((NCORES, L, P_MAX), dtype=np.float32)
    tmp[valid] = x[ids]
    INX[:, 0:P_MAX, :] = tmp.transpose(0, 2, 1)
    INX[:, P_MAX, :] = valid
    return INX


def _host_weights(type_embed, source_embed, W1, b1, W2, b2, Wf, bf, pairs):
    f = np.float32
    W1 = W1.astype(f); b1 = b1.astype(f); W2 = W2.astype(np.float64)
    b2 = b2.astype(np.float64); Wf = Wf.astype(np.float64); bf = bf.astype(np.float64)
    type_embed = type_embed.astype(np.float64)
    source_embed = source_embed.astype(np.float64)

    # layer1 lhsT blocks: [5, 9*256]; block t at cols [t*256,(t+1)*256)
    W1e = np.concatenate([W1, b1.astype(f)[:, None, :]], axis=1)   # [9,5,256]
    W1E = np.ascontiguousarray(W1e.transpose(1, 0, 2).reshape(KX, T * D))

    Wft, Wfs, Wfp = Wf[:D], Wf[D : 2 * D], Wf[2 * D :]

    # V[t] = W2[t] @ Wf_pv (f64), fusing layer 2 with the final projection.
    # bf16 lhsT blocks: [128, 18*256]; block (t,h) = V[t][h*128:(h+1)*128,:]
    V = (W2 @ Wfp).astype(f)                                        # [9,256,256]
    VR = np.ascontiguousarray(
        V.reshape(T, 2, 128, D).transpose(2, 0, 1, 3).reshape(128, T * 2 * D)
    ).astype(ml_dtypes.bfloat16)

    # bias table: C[pair=(t,tid,sid)] = te[tid]@Wft + se[sid]@Wfs + b2[t]@Wfp
    # + bf, stored transposed [128, 2*NPC] (half g at cols g*NPC + pcol)
    NPC = len(pairs)
    CG = np.zeros((128, 2 * NPC), dtype=f)
    for pcol in range(1, NPC):
        t, tid, sid = pairs[pcol]
        c = (type_embed[tid] @ Wft + source_embed[sid] @ Wfs
             + b2[t] @ Wfp + bf).astype(f)                          # [256]
        CG[:, pcol] = c[0:128]
        CG[:, NPC + pcol] = c[128:256]
    return W1E, VR, CG


def _build_program(m_t: tuple, L: int, ranges: tuple, NPC: int):
    """One compiled SPMD program for the given segment/bias-run layout."""
    key = (m_t, L, ranges, NPC, _WARM_BURST)
    if key in _PROGRAM_CACHE:
        return _PROGRAM_CACHE[key]

    group_expert = np.repeat(np.arange(T), (np.asarray(m_t) // GRP))
    NB = L // BLOCK
    GP = BLOCK // GRP  # groups per block = 4

    nc = bacc.Bacc("TRN2", target_bir_lowering=False, debug=False,
                   num_devices=NCORES)
    inx_d = nc.dram_tensor("inx", [KX, L], _F32R, kind="ExternalInput")
    w1e_d = nc.dram_tensor("w1e", [KX, T * D], _F32R, kind="ExternalInput")
    vr_d = nc.dram_tensor("vr", [128, T * 2 * D], _BF16, kind="ExternalInput")
    cg_d = nc.dram_tensor("cg", [128, 2 * NPC], _F32, kind="ExternalInput")
    out_d = nc.dram_tensor("out", [D, L], _BF16, kind="ExternalOutput")

    RELU = mybir.ActivationFunctionType.Relu
    ADD = mybir.AluOpType.add
    MAX = mybir.AluOpType.max

    with tile.TileContext(nc) as tc:
        with (
            tc.tile_pool(name="wts", bufs=1) as wts,
            tc.tile_pool(name="inp", bufs=1) as inp,
            tc.tile_pool(name="hsb", bufs=6) as hsbp,
            tc.tile_pool(name="osb", bufs=6) as osbp,
            tc.tile_pool(name="hps", bufs=4, space=bass.MemorySpace.PSUM) as hps,
            tc.tile_pool(name="ops", bufs=4, space=bass.MemorySpace.PSUM) as ops,
        ):
            # HAM warm-up burst first: tiny memsets, then bf16 matmuls that
            # overlap the input/weight DMAs and un-throttle the PE clock
            if _WARM_BURST:
                wmw = wts.tile([128, 128], _BF16)
                wma = wts.tile([128, 256], _BF16)
                nc.vector.memset(wmw[:], 0.0)
                nc.vector.memset(wma[:], 0.0)
                wmp = hps.tile([GRP, BLOCK], _F32, name="warmps", tag="hts")
                for i in range(_WARM_BURST):
                    nc.tensor.matmul(wmp[:, 0:256], wmw[:], wma[:], start=True,
                                     stop=True)

            w1e = wts.tile([128, T * D], _F32R)
            vr = wts.tile([128, T * 2 * D], _BF16)
            cg = wts.tile([128, 2 * NPC], _F32)
            # zero-fill w1e, then land the 5 valid rows on top, so layer-1
            # runs as a full 128-row fp32r matmul (partition slices must be
            # 32-aligned, so pad-only memsets are not expressible)
            nc.gpsimd.memset(w1e[:].bitcast(_F32), 0.0)
            nc.gpsimd.dma_start(w1e[0:KX, :], w1e_d.ap())
            nc.gpsimd.dma_start(cg[:], cg_d.ap())
            # V (1.13MB bf16) on the sync queue, idle until the first stores
            nc.sync.dma_start(vr[:], vr_d.ap())

            # persistent input buffers; zero-pad rows [KX:128) once so the
            # layer-1 matmul can run full-row (pads split over two engines)
            NIB = min(4, NB)
            xts = [inp.tile([128, BLOCK], _F32R, name=f"xtile{j}", tag=f"xtile{j}")
                   for j in range(NIB)]
            for j in range(NIB):
                eng = nc.vector if j % 2 == 0 else nc.gpsimd
                eng.memset(xts[j][:].bitcast(_F32), 0.0)

            for b in range(min(2, NIB)):
                nc.gpsimd.dma_start(
                    xts[b][0:KX, :], inx_d.ap()[:, b * BLOCK : (b + 1) * BLOCK])

            # bias runs clipped per block
            block_ranges = [[] for _ in range(NB)]
            for (c0, c1, pcol) in ranges:
                b0, b1 = c0 // BLOCK, (c1 - 1) // BLOCK
                for b in range(b0, b1 + 1):
                    r0 = max(c0, b * BLOCK) - b * BLOCK
                    r1 = min(c1, (b + 1) * BLOCK) - b * BLOCK
                    block_ranges[b].append((r0, r1, pcol))

            evac_rr = 0  # round-robin bias-relu ops across Scalar/Vector
            for b in range(NB):
                g0 = b * GP
                experts = [int(group_expert[g0 + i]) for i in range(GP)]
                runs = []
                for i, t in enumerate(experts):
                    if runs and runs[-1][0] == t:
                        runs[-1] = (t, runs[-1][1], (i + 1) * GRP)
                    else:
                        runs.append((t, i * GRP, (i + 1) * GRP))

                xt = xts[b % NIB]
                if b >= 2:
                    nc.gpsimd.dma_start(
                        xt[0:KX, :], inx_d.ap()[:, b * BLOCK : (b + 1) * BLOCK])

                # ---- layer 1: hT[h] = relu(W1e[t].T @ xT1), fp32r ----
                hts = [hps.tile([GRP, BLOCK], _F32, name=f"hts{b}_{j}", tag="hts")
                       for j in range(2)]
                for (t, c0, c1) in runs:
                    for h in range(2):
                        nc.tensor.matmul(
                            hts[h][:, c0:c1],
                            w1e[:, t * D + h * GRP : t * D + (h + 1) * GRP],
                            xt[:, c0:c1],
                            start=True, stop=True,
                        )
                hsb = [hsbp.tile([GRP, BLOCK], _BF16, name=f"hsb{b}_{j}", tag="hsb")
                       for j in range(2)]
                nc.scalar.activation(hsb[0][:], hts[0][:], RELU)
                nc.vector.tensor_scalar_max(hsb[1][:], hts[1][:], 0.0)

                # ---- V stage: outT[g] += V[t]-chunk(h).T @ hT[h], bf16 ----
                ots = [ops.tile([GRP, BLOCK], _F32, name=f"ots{b}_{j}", tag="ots")
                       for j in range(2)]
                for (t, c0, c1) in runs:
                    for h in range(2):
                        for g in range(2):
                            nc.tensor.matmul(
                                ots[g][:, c0:c1],
                                vr[:, (t * 2 + h) * D + g * GRP
                                   : (t * 2 + h) * D + (g + 1) * GRP],
                                hsb[h][:, c0:c1],
                                start=(h == 0), stop=(h == 1),
                            )

                # ---- bias-relu evacuation: out = relu(psum + C[pair]) ----
                osb = [osbp.tile([GRP, BLOCK], _BF16, name=f"osb{b}_{j}", tag="osb")
                       for j in range(2)]
                for (r0, r1, pcol) in block_ranges[b]:
                    for g in range(2):
                        bias_ap = cg[:, g * NPC + pcol : g * NPC + pcol + 1]
                        if evac_rr % 2 == 0:
                            nc.scalar.activation(
                                osb[g][:, r0:r1], ots[g][:, r0:r1], RELU,
                                bias=bias_ap)
                        else:
                            nc.vector.tensor_scalar(
                                osb[g][:, r0:r1], ots[g][:, r0:r1],
                                bias_ap, 0.0, op0=ADD, op1=MAX)
                        evac_rr += 1
                for g in range(2):
                    nc.sync.dma_start(
                        out_d.ap()[g * GRP : (g + 1) * GRP,
                                   b * BLOCK : (b + 1) * BLOCK],
                        osb[g][:],
                    )

    nc.compile()
    _PROGRAM_CACHE[key] = nc
    return nc


def kernel(type_ids, source_ids, params, type_embed, source_embed,
           W1, b1, W2, b2, Wf, bf):
    global LAST_RESULT
    type_ids = np.asarray(type_ids, dtype=np.int32)
    source_ids = np.asarray(source_ids, dtype=np.int32)
    params = np.asarray(params, dtype=np.float32)
    E = type_ids.shape[0]

    ORD, m_t, ranges, pairs, L = _layout(type_ids, source_ids)
    INX = _host_inputs(type_ids, source_ids, params, ORD)
    W1E, VR, CG = _host_weights(
        np.asarray(type_embed), np.asarray(source_embed),
        np.asarray(W1), np.asarray(b1), np.asarray(W2), np.asarray(b2),
        np.asarray(Wf), np.asarray(bf), pairs)

    nc = _build_program(tuple(int(v) for v in m_t), L,
                        tuple(ranges), len(pairs))

    in_maps = [{"inx": np.ascontiguousarray(INX[c]), "w1e": W1E, "vr": VR,
                "cg": CG} for c in range(NCORES)]

    trace = bool(int(os.environ.get("EDGEENC_TRACE", "0")))
    res = run_bass_kernel_spmd(nc, in_maps, core_ids=list(range(NCORES)),
                               trace=trace)
    LAST_RESULT = res

    full = np.zeros((E, D), dtype=np.float32)
    for c in range(NCORES):
        sel = ORD[c] >= 0
        oc = res.results[c]["out"]                     # [D, L] bf16
        full[ORD[c][sel]] = np.ascontiguousarray(
            oc[:, sel].T).astype(np.float32)
    return full


# revision 13
# speedup vs baseline: 1.2187x; 1.0453x over previous
"""Trainium2 Bass kernel for nn_EdgeEncoder (moe_routing).

Strategy
--------
Each of E edges is routed to 1 of 9 expert MLPs (4 -> 256 -> 256), then
  out = relu(concat([type_embed[tid], source_embed[sid], pv]) @ Wf + bf).

Host (numpy, cheap O(E) work):
  * scale/mask params; group edges by expert (base type), and within an
    expert by (type_id, source_id) pair; DEAL each pair's edges round-robin
    across the 8 cores so every core sees identical pair-run boundaries
    (ceil(n_pair/8) columns each) -> one compiled program serves all cores,
  * algebraic fusions so the device does minimal work:
      - x gets a ones-row so b1 rides inside the layer-1 matmul,
      - V[t] = W2[t] @ Wf_pv (f64 host precompute) fuses layer 2 with the
        final projection: pv @ Wf_pv == h @ V[t] + const,
      - the whole "embedding" term  C[tid,sid] = type_embed[tid] @ Wf_t
        + source_embed[sid] @ Wf_s + b2[t] @ Wf_pv + bf  is constant per
        (tid,sid) pair.  Because edges are sorted by pair, C is piecewise
        constant along the edge axis and rides FOR FREE as the per-partition
        bias operand of the PSUM->SBUF relu evacuation ops.  This deletes
        the K=20 one-hot matmuls entirely (8 -> 6 PE passes per block).

Device per 512-edge block (edges pre-grouped, transposed):
  hT  = relu(W1e[t].T @ xT1)           2 matmuls fp32r K=5(pad 128) N=512
  outT[g] += V[t]-chunk(h).T @ hT[h]   4 matmuls bf16  K=128        N=512
  relu(outT + C[pair]) PSUM -> bf16 SBUF (bias-relu ops split between the
  Scalar and Vector engines, one slice per pair-run), DMA outT tiles to
  DRAM [D, L] bf16; host un-permutes and upcasts.

All matmuls are full 128-row (inputs zero-padded): partial-K matmuls run
in PE tile mode and starve the HAM activity monitor, dropping the PE to
1.2 GHz.  A short bf16 warm-up burst raises the HAM clock gate at kernel
start while the first DMAs are in flight.
"""

import math
import os

import ml_dtypes
import numpy as np

import concourse.bacc as bacc
import concourse.bass as bass
import concourse.mybir as mybir
import concourse.tile as tile
from concourse.bass_utils import run_bass_kernel_spmd

# ---- static module configuration (mirrors the torch source) ----
T = 9            # base types ("experts")
P_MAX = 4
D = 256
N_TYPES = 14
N_SRC = 5
NCORES = 8
BLOCK = 512      # edges per device block (one PSUM bank of fp32)
GRP = 128        # expert segment granularity (PE partition dim)

BASE_MAP = np.array([0, 0, 0, 1, 1, 1, 2, 2, 3, 4, 5, 6, 7, 8], dtype=np.int32)
PCOUNT = np.array([2, 2, 1, 1, 1, 1, 3, 2, 4], dtype=np.int32)
SCALES = np.ones((T, P_MAX), dtype=np.float32)
SCALES[0, :2] = [1.0, 1e-06]      # nmos  m, w
SCALES[1, :2] = [1.0, 1e-06]      # pmos  m, w
SCALES[2, 0] = 1.0                # balun rout
SCALES[3, 0] = 1000.0             # resistor r
SCALES[4, 0] = 1e-12              # capacitor c
SCALES[5, 0] = 1e-09              # inductor l
SCALES[6, :3] = [1.0, 1.0, 1.0]   # vsource dc, mag, phase
SCALES[7, :2] = [0.001, 0.001]    # isource dc, mag
SCALES[8, :4] = [1.0, 1.0, 1e9, 1.0]  # port dbm, dc, freq, num

KX = 5                            # x rows: xT(4) + ones
_F32 = mybir.dt.float32
_F32R = mybir.dt.float32r
_BF16 = mybir.dt.bfloat16
_WARM_BURST = int(os.environ.get("EDGEENC_WARM_BURST", "10"))

_PROGRAM_CACHE: dict = {}
LAST_RESULT = None  # BassKernelResults of the most recent run (for test harness)


def _layout(type_ids, source_ids):
    """Sort edges by (expert, type, source); deal each (t,tid,sid) group
    round-robin over cores.  Returns:
      ORD    [NCORES, L] global edge index per slot (-1 = pad)
      m_t    [T] per-core expert segment sizes (multiples of GRP)
      ranges list of (c0, c1, pair_col) bias runs, identical on all cores
      pairs  list of (t, tid, sid) in pair_col order (col 0 = zero bias)
      L      padded per-core column count (multiple of BLOCK)
    """
    base_ids = BASE_MAP[type_ids]
    m_t = np.zeros(T, dtype=np.int64)
    cols = []          # per expert: list of (k_p, pair_id, idx array)
    pairs = [None]     # pair_col 0 reserved for zero bias (pad columns)
    ranges = []
    ORD_parts = []
    off = 0
    for t in range(T):
        sel = np.nonzero(base_ids == t)[0]
        if sel.shape[0] == 0:
            continue
        key = type_ids[sel].astype(np.int64) * N_SRC + source_ids[sel]
        order = np.argsort(key, kind="stable")
        sel = sel[order]
        key = key[order]
        uk, counts = np.unique(key, return_counts=True)
        seg = 0
        pos = 0
        ord_t = []
        for u, n in zip(uk, counts):
            k_p = math.ceil(n / NCORES)
            pcol = len(pairs)
            pairs.append((t, int(u) // N_SRC, int(u) % N_SRC))
            idx = np.full(NCORES * k_p, -1, dtype=np.int64)
            idx[:n] = sel[pos : pos + n]
            pos += n
            # slot j of core c <- idx[j*NCORES + c]
            ord_t.append(idx.reshape(k_p, NCORES).T)     # [NCORES, k_p]
            ranges.append((off + seg, off + seg + k_p, pcol))
            seg += k_p
        m = math.ceil(seg / GRP) * GRP
        if m > seg:
            ranges.append((off + seg, off + m, 0))
            ord_t.append(np.full((NCORES, m - seg), -1, dtype=np.int64))
        m_t[t] = m
        ORD_parts.append(np.concatenate(ord_t, axis=1))
        off += m
    L0 = off
    L = math.ceil(L0 / BLOCK) * BLOCK
    if L > L0:
        ranges.append((L0, L, 0))
        ORD_parts.append(np.full((NCORES, L - L0), -1, dtype=np.int64))
        m_t[np.nonzero(m_t)[0][-1]] += L - L0
    ORD = np.concatenate(ORD_parts, axis=1)
    return ORD, m_t, ranges, pairs, L


def _host_inputs(type_ids, source_ids, params, ORD):
    """INX[c] = [5, L] bf16: xT (scaled/masked) + ones row."""
    base_ids = BASE_MAP[type_ids]
    scales = SCALES[base_ids]                                  # [E,4]
    validp = np.arange(P_MAX)[None, :] < PCOUNT[base_ids][:, None]
    x = np.where(validp, params.astype(np.float32) / scales, 0.0).astype(np.float32)

    L = ORD.shape[1]
    INX = np.zeros((NCORES, KX, L), dtype=np.float32)
    valid = ORD >= 0
    ids = ORD[valid]
    tmp = np.zeros((NCORES, L, P_MAX), dtype=np.float32)
    tmp[valid] = x[ids]
    INX[:, 0:P_MAX, :] = tmp.transpose(0, 2, 1)
    INX[:, P_MAX, :] = valid
    return INX.astype(ml_dtypes.bfloat16)


def _host_weights(type_embed, source_embed, W1, b1, W2, b2, Wf, bf, pairs):
    f = np.float32
    W1 = W1.astype(f); b1 = b1.astype(f); W2 = W2.astype(np.float64)
    b2 = b2.astype(np.float64); Wf = Wf.astype(np.float64); bf = bf.astype(np.float64)
    type_embed = type_embed.astype(np.float64)
    source_embed = source_embed.astype(np.float64)

    # layer1 lhsT blocks: [5, 9*256]; block t at cols [t*256,(t+1)*256)
    W1e = np.concatenate([W1, b1.astype(f)[:, None, :]], axis=1)   # [9,5,256]
    W1E = np.ascontiguousarray(
        W1e.transpose(1, 0, 2).reshape(KX, T * D)).astype(ml_dtypes.bfloat16)

    Wft, Wfs, Wfp = Wf[:D], Wf[D : 2 * D], Wf[2 * D :]

    # V[t] = W2[t] @ Wf_pv (f64), fusing layer 2 with the final projection.
    # bf16 lhsT blocks: [128, 18*256]; block (t,h) = V[t][h*128:(h+1)*128,:]
    V = (W2 @ Wfp).astype(f)                                        # [9,256,256]
    VR = np.ascontiguousarray(
        V.reshape(T, 2, 128, D).transpose(2, 0, 1, 3).reshape(128, T * 2 * D)
    ).astype(ml_dtypes.bfloat16)

    # bias table: C[pair=(t,tid,sid)] = te[tid]@Wft + se[sid]@Wfs + b2[t]@Wfp
    # + bf, stored transposed [128, 2*NPC] (half g at cols g*NPC + pcol)
    NPC = len(pairs)
    CG = np.zeros((128, 2 * NPC), dtype=f)
    for pcol in range(1, NPC):
        t, tid, sid = pairs[pcol]
        c = (type_embed[tid] @ Wft + source_embed[sid] @ Wfs
             + b2[t] @ Wfp + bf).astype(f)                          # [256]
        CG[:, pcol] = c[0:128]
        CG[:, NPC + pcol] = c[128:256]
    return W1E, VR, CG


def _build_program(m_t: tuple, L: int, ranges: tuple, NPC: int):
    """One compiled SPMD program for the given segment/bias-run layout."""
    key = (m_t, L, ranges, NPC, _WARM_BURST)
    if key in _PROGRAM_CACHE:
        return _PROGRAM_CACHE[key]

    group_expert = np.repeat(np.arange(T), (np.asarray(m_t) // GRP))
    NB = L // BLOCK
    GP = BLOCK // GRP  # groups per block = 4

    nc = bacc.Bacc("TRN2", target_bir_lowering=False, debug=False,
                   num_devices=NCORES)
    inx_d = nc.dram_tensor("inx", [KX, L], _BF16, kind="ExternalInput")
    w1e_d = nc.dram_tensor("w1e", [KX, T * D], _BF16, kind="ExternalInput")
    vr_d = nc.dram_tensor("vr", [128, T * 2 * D], _BF16, kind="ExternalInput")
    cg_d = nc.dram_tensor("cg", [128, 2 * NPC], _F32, kind="ExternalInput")
    out_d = nc.dram_tensor("out", [D, L], _BF16, kind="ExternalOutput")

    RELU = mybir.ActivationFunctionType.Relu
    ADD = mybir.AluOpType.add
    MAX = mybir.AluOpType.max

    with tile.TileContext(nc) as tc:
        with (
            tc.tile_pool(name="wts", bufs=1) as wts,
            tc.tile_pool(name="inp", bufs=1) as inp,
            tc.tile_pool(name="hsb", bufs=6) as hsbp,
            tc.tile_pool(name="osb", bufs=6) as osbp,
            tc.tile_pool(name="hps", bufs=4, space=bass.MemorySpace.PSUM) as hps,
            tc.tile_pool(name="ops", bufs=4, space=bass.MemorySpace.PSUM) as ops,
        ):
            # HAM warm-up burst first: tiny memsets, then bf16 matmuls that
            # overlap the input/weight DMAs and un-throttle the PE clock
            if _WARM_BURST:
                wmw = wts.tile([128, 128], _BF16)
                wma = wts.tile([128, 256], _BF16)
                nc.vector.memset(wmw[:], 0.0)
                nc.vector.memset(wma[:], 0.0)
                wmp = hps.tile([GRP, BLOCK], _F32, name="warmps", tag="hts")
                for i in range(_WARM_BURST):
                    nc.tensor.matmul(wmp[:, 0:256], wmw[:], wma[:], start=True,
                                     stop=True)

            w1e = wts.tile([128, T * D], _BF16)
            vr = wts.tile([128, T * 2 * D], _BF16)
            cg = wts.tile([128, 2 * NPC], _F32)
            # zero-fill w1e, then land the 5 valid rows on top, so layer-1
            # runs as a full 128-row matmul (partition slices must be
            # 32-aligned, so pad-only memsets are not expressible)
            nc.gpsimd.memset(w1e[:], 0.0)
            nc.gpsimd.dma_start(w1e[0:KX, :], w1e_d.ap())
            nc.gpsimd.dma_start(cg[:], cg_d.ap())
            # V (1.13MB bf16) split across the sync + scalar queues (both
            # otherwise idle at start) so the first V matmuls aren't stalled
            HALFV = T * D  # = half of the vr columns
            nc.sync.dma_start(vr[:, 0:HALFV], vr_d.ap()[:, 0:HALFV])
            nc.scalar.dma_start(vr[:, HALFV:], vr_d.ap()[:, HALFV:])

            # persistent input buffers; zero-pad rows [KX:128) once so the
            # layer-1 matmul can run full-row (pads split over two engines)
            NIB = min(8, NB)
            PREF = min(4, NIB)
            xts = [inp.tile([128, BLOCK], _BF16, name=f"xtile{j}", tag=f"xtile{j}")
                   for j in range(NIB)]
            for j in range(NIB):
                eng = nc.vector if j % 2 == 0 else nc.gpsimd
                eng.memset(xts[j][:], 0.0)

            for b in range(min(PREF, NB)):
                nc.gpsimd.dma_start(
                    xts[b][0:KX, :], inx_d.ap()[:, b * BLOCK : (b + 1) * BLOCK])

            # bias runs clipped per block
            block_ranges = [[] for _ in range(NB)]
            for (c0, c1, pcol) in ranges:
                b0, b1 = c0 // BLOCK, (c1 - 1) // BLOCK
                for b in range(b0, b1 + 1):
                    r0 = max(c0, b * BLOCK) - b * BLOCK
                    r1 = min(c1, (b + 1) * BLOCK) - b * BLOCK
                    block_ranges[b].append((r0, r1, pcol))

            evac_rr = 0  # round-robin bias-relu ops across Scalar/Vector
            for b in range(NB):
                g0 = b * GP
                experts = [int(group_expert[g0 + i]) for i in range(GP)]
                runs = []
                for i, t in enumerate(experts):
                    if runs and runs[-1][0] == t:
                        runs[-1] = (t, runs[-1][1], (i + 1) * GRP)
                    else:
                        runs.append((t, i * GRP, (i + 1) * GRP))

                xt = xts[b % NIB]
                bp = b + PREF
                if bp < NB:
                    xtp = xts[bp % NIB]
                    nc.gpsimd.dma_start(
                        xtp[0:KX, :],
                        inx_d.ap()[:, bp * BLOCK : (bp + 1) * BLOCK])

                # ---- layer 1: hT[h] = relu(W1e[t].T @ xT1), fp32r ----
                hts = [hps.tile([GRP, BLOCK], _F32, name=f"hts{b}_{j}", tag="hts")
                       for j in range(2)]
                for (t, c0, c1) in runs:
                    for h in range(2):
                        nc.tensor.matmul(
                            hts[h][:, c0:c1],
                            w1e[:, t * D + h * GRP : t * D + (h + 1) * GRP],
                            xt[:, c0:c1],
                            start=True, stop=True,
                        )
                hsb = [hsbp.tile([GRP, BLOCK], _BF16, name=f"hsb{b}_{j}", tag="hsb")
                       for j in range(2)]
                nc.scalar.activation(hsb[0][:], hts[0][:], RELU)
                nc.vector.tensor_scalar_max(hsb[1][:], hts[1][:], 0.0)

                # ---- V stage: outT[g] += V[t]-chunk(h).T @ hT[h], bf16 ----
                ots = [ops.tile([GRP, BLOCK], _F32, name=f"ots{b}_{j}", tag="ots")
                       for j in range(2)]
                for (t, c0, c1) in runs:
                    for h in range(2):
                        for g in range(2):
                            nc.tensor.matmul(
                                ots[g][:, c0:c1],
                                vr[:, (t * 2 + h) * D + g * GRP
                                   : (t * 2 + h) * D + (g + 1) * GRP],
                                hsb[h][:, c0:c1],
                                start=(h == 0), stop=(h == 1),
                            )

                # ---- bias-relu evacuation: out = relu(psum + C[pair]) ----
                osb = [osbp.tile([GRP, BLOCK], _BF16, name=f"osb{b}_{j}", tag="osb")
                       for j in range(2)]
                for (r0, r1, pcol) in block_ranges[b]:
                    for g in range(2):
                        bias_ap = cg[:, g * NPC + pcol : g * NPC + pcol + 1]
                        if evac_rr % 2 == 0:
                            nc.scalar.activation(
                                osb[g][:, r0:r1], ots[g][:, r0:r1], RELU,
                                bias=bias_ap)
                        else:
                            nc.vector.tensor_scalar(
                                osb[g][:, r0:r1], ots[g][:, r0:r1],
                                bias_ap, 0.0, op0=ADD, op1=MAX)
                        evac_rr += 1
                for g in range(2):
                    nc.sync.dma_start(
                        out_d.ap()[g * GRP : (g + 1) * GRP,
                                   b * BLOCK : (b + 1) * BLOCK],
                        osb[g][:],
                    )

    nc.compile()
    _PROGRAM_CACHE[key] = nc
    return nc


def kernel(type_ids, source_ids, params, type_embed, source_embed,
           W1, b1, W2, b2, Wf, bf):
    global LAST_RESULT
    type_ids = np.asarray(type_ids, dtype=np.int32)
    source_ids = np.asarray(source_ids, dtype=np.int32)
    params = np.asarray(params, dtype=np.float32)
    E = type_ids.shape[0]

    ORD, m_t, ranges, pairs, L = _layout(type_ids, source_ids)
    INX = _host_inputs(type_ids, source_ids, params, ORD)
    W1E, VR, CG = _host_weights(
        np.asarray(type_embed), np.asarray(source_embed),
        np.asarray(W1), np.asarray(b1), np.asarray(W2), np.asarray(b2),
        np.asarray(Wf), np.asarray(bf), pairs)

    nc = _build_program(tuple(int(v) for v in m_t), L,
                        tuple(ranges), len(pairs))

    in_maps = [{"inx": np.ascontiguousarray(INX[c]), "w1e": W1E, "vr": VR,
                "cg": CG} for c in range(NCORES)]

    trace = bool(int(os.environ.get("EDGEENC_TRACE", "0")))
    res = run_bass_kernel_spmd(nc, in_maps, core_ids=list(range(NCORES)),
                               trace=trace)
    LAST_RESULT = res

    full = np.zeros((E, D), dtype=np.float32)
    for c in range(NCORES):
        sel = ORD[c] >= 0
        oc = res.results[c]["out"]                     # [D, L] bf16
        full[ORD[c][sel]] = np.ascontiguousarray(
            oc[:, sel].T).astype(np.float32)
    return full


# revision 21
# speedup vs baseline: 1.2392x; 1.0168x over previous
"""Trainium2 Bass kernel for nn_EdgeEncoder (moe_routing).

Strategy
--------
Each of E edges is routed to 1 of 9 expert MLPs (4 -> 256 -> 256), then
  out = relu(concat([type_embed[tid], source_embed[sid], pv]) @ Wf + bf).

Host (numpy, cheap O(E) work):
  * scale/mask params; group edges by expert (base type), and within an
    expert by (type_id, source_id) pair; DEAL each pair's edges round-robin
    across the 8 cores so every core sees identical pair-run boundaries
    (ceil(n_pair/8) columns each) -> one compiled program serves all cores,
  * algebraic fusions so the device does minimal work:
      - x gets a ones-row so b1 rides inside the layer-1 matmul,
      - V[t] = W2[t] @ Wf_pv (f64 host precompute) fuses layer 2 with the
        final projection: pv @ Wf_pv == h @ V[t] + const,
      - the whole "embedding" term  C[tid,sid] = type_embed[tid] @ Wf_t
        + source_embed[sid] @ Wf_s + b2[t] @ Wf_pv + bf  is constant per
        (tid,sid) pair.  Because edges are sorted by pair, C is piecewise
        constant along the edge axis and rides FOR FREE as the per-partition
        bias operand of the PSUM->SBUF relu evacuation ops.  This deletes
        the K=20 one-hot matmuls entirely (8 -> 6 PE passes per block).

Device per 512-edge block (edges pre-grouped, transposed):
  hT  = relu(W1e[t].T @ xT1)           2 matmuls fp32r K=5(pad 128) N=512
  outT[g] += V[t]-chunk(h).T @ hT[h]   4 matmuls bf16  K=128        N=512
  relu(outT + C[pair]) PSUM -> bf16 SBUF (bias-relu ops split between the
  Scalar and Vector engines, one slice per pair-run), DMA outT tiles to
  DRAM [D, L] bf16; host un-permutes and upcasts.

All matmuls are full 128-row (inputs zero-padded): partial-K matmuls run
in PE tile mode and starve the HAM activity monitor, dropping the PE to
1.2 GHz.  A short bf16 warm-up burst raises the HAM clock gate at kernel
start while the first DMAs are in flight.
"""

import math
import os

import ml_dtypes
import numpy as np

import concourse.bacc as bacc
import concourse.bass as bass
import concourse.mybir as mybir
import concourse.tile as tile
from concourse.bass_utils import run_bass_kernel_spmd

# ---- static module configuration (mirrors the torch source) ----
T = 9            # base types ("experts")
P_MAX = 4
D = 256
N_TYPES = 14
N_SRC = 5
NCORES = 8
BLOCK = 512      # edges per device block (one PSUM bank of fp32)
GRP = 8          # expert segment granularity (PSUM 8-byte-line alignment)
PDIM = 128       # PE partition dim

BASE_MAP = np.array([0, 0, 0, 1, 1, 1, 2, 2, 3, 4, 5, 6, 7, 8], dtype=np.int32)
PCOUNT = np.array([2, 2, 1, 1, 1, 1, 3, 2, 4], dtype=np.int32)
SCALES = np.ones((T, P_MAX), dtype=np.float32)
SCALES[0, :2] = [1.0, 1e-06]      # nmos  m, w
SCALES[1, :2] = [1.0, 1e-06]      # pmos  m, w
SCALES[2, 0] = 1.0                # balun rout
SCALES[3, 0] = 1000.0             # resistor r
SCALES[4, 0] = 1e-12              # capacitor c
SCALES[5, 0] = 1e-09              # inductor l
SCALES[6, :3] = [1.0, 1.0, 1.0]   # vsource dc, mag, phase
SCALES[7, :2] = [0.001, 0.001]    # isource dc, mag
SCALES[8, :4] = [1.0, 1.0, 1e9, 1.0]  # port dbm, dc, freq, num

KX = 5                            # x rows: xT(4) + ones
_F32 = mybir.dt.float32
_F32R = mybir.dt.float32r
_BF16 = mybir.dt.bfloat16
_WARM_BURST = int(os.environ.get("EDGEENC_WARM_BURST", "10"))

_PROGRAM_CACHE: dict = {}
LAST_RESULT = None  # BassKernelResults of the most recent run (for test harness)


def _layout(type_ids, source_ids):
    """Sort edges by (expert, type, source); deal each (t,tid,sid) group
    round-robin over cores.  Returns:
      ORD    [NCORES, L] global edge index per slot (-1 = pad)
      m_t    [T] per-core expert segment sizes (multiples of GRP)
      ranges list of (c0, c1, pair_col) bias runs, identical on all cores
             (pair_col 0 = pad columns, skipped by the device entirely)
      pairs  list of (t, tid, sid) in pair_col order (col 0 = zero bias)
      L      padded per-core column count (multiple of BLOCK)
    """
    base_ids = BASE_MAP[type_ids]
    m_t = np.zeros(T, dtype=np.int64)
    cols = []          # per expert: list of (k_p, pair_id, idx array)
    pairs = [None]     # pair_col 0 reserved for zero bias (pad columns)
    ranges = []
    ORD_parts = []
    off = 0
    for t in range(T):
        sel = np.nonzero(base_ids == t)[0]
        if sel.shape[0] == 0:
            continue
        key = type_ids[sel].astype(np.int64) * N_SRC + source_ids[sel]
        order = np.argsort(key, kind="stable")
        sel = sel[order]
        key = key[order]
        uk, counts = np.unique(key, return_counts=True)
        seg = 0
        pos = 0
        ord_t = []
        for u, n in zip(uk, counts):
            k_p = math.ceil(n / NCORES)
            pcol = len(pairs)
            pairs.append((t, int(u) // N_SRC, int(u) % N_SRC))
            idx = np.full(NCORES * k_p, -1, dtype=np.int64)
            idx[:n] = sel[pos : pos + n]
            pos += n
            # slot j of core c <- idx[j*NCORES + c]
            ord_t.append(idx.reshape(k_p, NCORES).T)     # [NCORES, k_p]
            ranges.append((off + seg, off + seg + k_p, pcol))
            seg += k_p
        m = math.ceil(seg / GRP) * GRP
        if m > seg:
            ranges.append((off + seg, off + m, 0))
            ord_t.append(np.full((NCORES, m - seg), -1, dtype=np.int64))
        m_t[t] = m
        ORD_parts.append(np.concatenate(ord_t, axis=1))
        off += m
    L0 = off
    L = math.ceil(L0 / BLOCK) * BLOCK
    if L > L0:
        ranges.append((L0, L, 0))
        ORD_parts.append(np.full((NCORES, L - L0), -1, dtype=np.int64))
        m_t[np.nonzero(m_t)[0][-1]] += L - L0
    ORD = np.concatenate(ORD_parts, axis=1)
    return ORD, m_t, ranges, pairs, L


def _host_inputs(type_ids, source_ids, params, ORD):
    """INX[c] = [5, L] bf16: xT (scaled/masked) + ones row."""
    base_ids = BASE_MAP[type_ids]
    scales = SCALES[base_ids]                                  # [E,4]
    validp = np.arange(P_MAX)[None, :] < PCOUNT[base_ids][:, None]
    x = np.where(validp, params.astype(np.float32) / scales, 0.0).astype(np.float32)

    L = ORD.shape[1]
    INX = np.zeros((NCORES, KX, L), dtype=np.float32)
    valid = ORD >= 0
    ids = ORD[valid]
    tmp = np.zeros((NCORES, L, P_MAX), dtype=np.float32)
    tmp[valid] = x[ids]
    INX[:, 0:P_MAX, :] = tmp.transpose(0, 2, 1)
    INX[:, P_MAX, :] = valid
    return INX.astype(ml_dtypes.bfloat16)


def _host_weights(type_embed, source_embed, W1, b1, W2, b2, Wf, bf, pairs):
    f = np.float32
    W1 = W1.astype(f); b1 = b1.astype(f); W2 = W2.astype(np.float64)
    b2 = b2.astype(np.float64); Wf = Wf.astype(np.float64); bf = bf.astype(np.float64)
    type_embed = type_embed.astype(np.float64)
    source_embed = source_embed.astype(np.float64)

    # layer1 lhsT blocks: [5, 9*256]; block t at cols [t*256,(t+1)*256)
    W1e = np.concatenate([W1, b1.astype(f)[:, None, :]], axis=1)   # [9,5,256]
    W1E = np.ascontiguousarray(
        W1e.transpose(1, 0, 2).reshape(KX, T * D)).astype(ml_dtypes.bfloat16)

    Wft, Wfs, Wfp = Wf[:D], Wf[D : 2 * D], Wf[2 * D :]

    # V[t] = W2[t] @ Wf_pv (f64), fusing layer 2 with the final projection.
    # bf16 lhsT blocks: [128, 18*256]; block (t,h) = V[t][h*128:(h+1)*128,:]
    V = (W2 @ Wfp).astype(f)                                        # [9,256,256]
    VR = np.ascontiguousarray(
        V.reshape(T, 2, 128, D).transpose(2, 0, 1, 3).reshape(128, T * 2 * D)
    ).astype(ml_dtypes.bfloat16)

    # bias table: C[pair=(t,tid,sid)] = te[tid]@Wft + se[sid]@Wfs + b2[t]@Wfp
    # + bf, stored transposed [128, 2*NPC] (half g at cols g*NPC + pcol)
    NPC = len(pairs)
    CG = np.zeros((128, 2 * NPC), dtype=f)
    for pcol in range(1, NPC):
        t, tid, sid = pairs[pcol]
        c = (type_embed[tid] @ Wft + source_embed[sid] @ Wfs
             + b2[t] @ Wfp + bf).astype(f)                          # [256]
        CG[:, pcol] = c[0:128]
        CG[:, NPC + pcol] = c[128:256]
    return W1E, VR, CG


def _build_program(m_t: tuple, L: int, ranges: tuple, NPC: int):
    """One compiled SPMD program for the given segment/bias-run layout."""
    key = (m_t, L, ranges, NPC, _WARM_BURST)
    if key in _PROGRAM_CACHE:
        return _PROGRAM_CACHE[key]

    group_expert = np.repeat(np.arange(T), (np.asarray(m_t) // GRP))
    NB = L // BLOCK
    GP = BLOCK // GRP  # groups per block = 4

    nc = bacc.Bacc("TRN2", target_bir_lowering=False, debug=False,
                   num_devices=NCORES)
    inx_d = nc.dram_tensor("inx", [KX, L], _BF16, kind="ExternalInput")
    w1e_d = nc.dram_tensor("w1e", [KX, T * D], _BF16, kind="ExternalInput")
    vr_d = nc.dram_tensor("vr", [128, T * 2 * D], _BF16, kind="ExternalInput")
    cg_d = nc.dram_tensor("cg", [128, 2 * NPC], _F32, kind="ExternalInput")
    out_d = nc.dram_tensor("out", [D, L], _BF16, kind="ExternalOutput")

    RELU = mybir.ActivationFunctionType.Relu
    ADD = mybir.AluOpType.add
    MAX = mybir.AluOpType.max

    with tile.TileContext(nc) as tc:
        with (
            tc.tile_pool(name="wts", bufs=1) as wts,
            tc.tile_pool(name="inp", bufs=1) as inp,
            tc.tile_pool(name="hsb", bufs=6) as hsbp,
            tc.tile_pool(name="osb", bufs=6) as osbp,
            tc.tile_pool(name="hps", bufs=4, space=bass.MemorySpace.PSUM) as hps,
            tc.tile_pool(name="ops", bufs=4, space=bass.MemorySpace.PSUM) as ops,
        ):
            # HAM warm-up burst first: tiny memsets, then bf16 matmuls that
            # overlap the input/weight DMAs and un-throttle the PE clock
            if _WARM_BURST:
                wmw = wts.tile([128, 128], _BF16)
                wma = wts.tile([128, 256], _BF16)
                nc.vector.memset(wmw[:], 0.0)
                nc.vector.memset(wma[:], 0.0)
                wmp = hps.tile([PDIM, BLOCK], _F32, name="warmps", tag="hts")
                for i in range(_WARM_BURST):
                    nc.tensor.matmul(wmp[:, 0:256], wmw[:], wma[:], start=True,
                                     stop=True)

            w1e = wts.tile([128, T * D], _BF16)
            vr = wts.tile([128, T * 2 * D], _BF16)
            cg = wts.tile([128, 2 * NPC], _F32)
            # zero-fill w1e, then land the 5 valid rows on top, so layer-1
            # runs as a full 128-row matmul (partition slices must be
            # 32-aligned, so pad-only memsets are not expressible)
            nc.gpsimd.memset(w1e[:], 0.0)
            nc.gpsimd.dma_start(w1e[0:KX, :], w1e_d.ap())
            nc.gpsimd.dma_start(cg[:], cg_d.ap())
            # V (1.13MB bf16): expert 0's chunk first (the first ~7 blocks
            # are all expert 0), remainder split across the sync + scalar
            # queues, so the first V matmuls aren't stalled on the full load
            HALFV = T * D  # = half of the vr columns
            nc.sync.dma_start(vr[:, 0 : 2 * D], vr_d.ap()[:, 0 : 2 * D])
            nc.sync.dma_start(vr[:, 2 * D : HALFV], vr_d.ap()[:, 2 * D : HALFV])
            nc.scalar.dma_start(vr[:, HALFV:], vr_d.ap()[:, HALFV:])

            # persistent input buffers; zero-pad rows [KX:128) once so the
            # layer-1 matmul can run full-row (pads split over two engines)
            NIB = min(8, NB)
            PREF = min(4, NIB)
            xts = [inp.tile([128, BLOCK], _BF16, name=f"xtile{j}", tag=f"xtile{j}")
                   for j in range(NIB)]
            for j in range(NIB):
                eng = nc.vector if j % 2 == 0 else nc.gpsimd
                eng.memset(xts[j][:], 0.0)

            for b in range(min(PREF, NB)):
                nc.gpsimd.dma_start(
                    xts[b][0:KX, :], inx_d.ap()[:, b * BLOCK : (b + 1) * BLOCK])

            # bias runs clipped per block; pad runs (pcol 0) are skipped and
            # also clipped out of the matmuls/evacuation/stores below
            block_ranges = [[] for _ in range(NB)]
            for (c0, c1, pcol) in ranges:
                if pcol == 0:
                    continue
                b0, b1 = c0 // BLOCK, (c1 - 1) // BLOCK
                for b in range(b0, b1 + 1):
                    r0 = max(c0, b * BLOCK) - b * BLOCK
                    r1 = min(c1, (b + 1) * BLOCK) - b * BLOCK
                    block_ranges[b].append((r0, r1, pcol))
            # pad groups: GRP-col groups lying fully inside a pad range
            is_pad = np.zeros(L // GRP, dtype=bool)
            for (c0, c1, pcol) in ranges:
                if pcol == 0:
                    g0p = (c0 + GRP - 1) // GRP
                    for g in range(g0p, c1 // GRP):
                        is_pad[g] = True

            evac_rr = 0  # round-robin bias-relu ops across Scalar/Vector
            for b in range(NB):
                g0 = b * GP
                experts = [int(group_expert[g0 + i]) for i in range(GP)]
                runs = []
                for i, t in enumerate(experts):
                    if runs and runs[-1][0] == t:
                        runs[-1] = (t, runs[-1][1], (i + 1) * GRP)
                    else:
                        runs.append((t, i * GRP, (i + 1) * GRP))
                # clip trailing pad groups out of each run (drop empty runs)
                cruns = []
                for (t, c0, c1) in runs:
                    while c1 > c0 and is_pad[g0 + c1 // GRP - 1]:
                        c1 -= GRP
                    if c1 > c0:
                        cruns.append((t, c0, c1))
                runs = cruns
                if not runs:
                    continue
                vend = max(c1 for (_, _, c1) in runs)

                xt = xts[b % NIB]
                bp = b + PREF
                if bp < NB:
                    xtp = xts[bp % NIB]
                    nc.gpsimd.dma_start(
                        xtp[0:KX, :],
                        inx_d.ap()[:, bp * BLOCK : (bp + 1) * BLOCK])

                # ---- layer 1: hT[h] = relu(W1e[t].T @ xT1), fp32r ----
                hts = [hps.tile([PDIM, BLOCK], _F32, name=f"hts{b}_{j}", tag="hts")
                       for j in range(2)]
                for (t, c0, c1) in runs:
                    for h in range(2):
                        nc.tensor.matmul(
                            hts[h][:, c0:c1],
                            w1e[:, t * D + h * PDIM : t * D + (h + 1) * PDIM],
                            xt[:, c0:c1],
                            start=True, stop=True,
                        )
                hsb = [hsbp.tile([PDIM, BLOCK], _BF16, name=f"hsb{b}_{j}", tag="hsb")
                       for j in range(2)]
                nc.scalar.activation(hsb[0][:, 0:vend], hts[0][:, 0:vend], RELU)
                nc.vector.tensor_scalar_max(hsb[1][:, 0:vend], hts[1][:, 0:vend],
                                            0.0)

                # ---- V stage: outT[g] += V[t]-chunk(h).T @ hT[h], bf16 ----
                ots = [ops.tile([PDIM, BLOCK], _F32, name=f"ots{b}_{j}", tag="ots")
                       for j in range(2)]
                for (t, c0, c1) in runs:
                    for h in range(2):
                        for g in range(2):
                            nc.tensor.matmul(
                                ots[g][:, c0:c1],
                                vr[:, (t * 2 + h) * D + g * PDIM
                                   : (t * 2 + h) * D + (g + 1) * PDIM],
                                hsb[h][:, c0:c1],
                                start=(h == 0), stop=(h == 1),
                            )

                # ---- bias-relu evacuation: out = relu(psum + C[pair]) ----
                osb = [osbp.tile([PDIM, BLOCK], _BF16, name=f"osb{b}_{j}", tag="osb")
                       for j in range(2)]
                for (r0, r1, pcol) in block_ranges[b]:
                    for g in range(2):
                        bias_ap = cg[:, g * NPC + pcol : g * NPC + pcol + 1]
                        if evac_rr % 2 == 0:
                            nc.scalar.activation(
                                osb[g][:, r0:r1], ots[g][:, r0:r1], RELU,
                                bias=bias_ap)
                        else:
                            nc.vector.tensor_scalar(
                                osb[g][:, r0:r1], ots[g][:, r0:r1],
                                bias_ap, 0.0, op0=ADD, op1=MAX)
                        evac_rr += 1
                for g in range(2):
                    nc.sync.dma_start(
                        out_d.ap()[g * PDIM : (g + 1) * PDIM,
                                   b * BLOCK : b * BLOCK + vend],
                        osb[g][:, 0:vend],
                    )

    nc.compile()
    _PROGRAM_CACHE[key] = nc
    return nc


def kernel(type_ids, source_ids, params, type_embed, source_embed,
           W1, b1, W2, b2, Wf, bf):
    global LAST_RESULT
    type_ids = np.asarray(type_ids, dtype=np.int32)
    source_ids = np.asarray(source_ids, dtype=np.int32)
    params = np.asarray(params, dtype=np.float32)
    E = type_ids.shape[0]

    ORD, m_t, ranges, pairs, L = _layout(type_ids, source_ids)
    INX = _host_inputs(type_ids, source_ids, params, ORD)
    W1E, VR, CG = _host_weights(
        np.asarray(type_embed), np.asarray(source_embed),
        np.asarray(W1), np.asarray(b1), np.asarray(W2), np.asarray(b2),
        np.asarray(Wf), np.asarray(bf), pairs)

    nc = _build_program(tuple(int(v) for v in m_t), L,
                        tuple(ranges), len(pairs))

    in_maps = [{"inx": np.ascontiguousarray(INX[c]), "w1e": W1E, "vr": VR,
                "cg": CG} for c in range(NCORES)]

    trace = bool(int(os.environ.get("EDGEENC_TRACE", "0")))
    res = run_bass_kernel_spmd(nc, in_maps, core_ids=list(range(NCORES)),
                               trace=trace)
    LAST_RESULT = res

    full = np.zeros((E, D), dtype=np.float32)
    for c in range(NCORES):
        sel = ORD[c] >= 0
        oc = res.results[c]["out"]                     # [D, L] bf16
        full[ORD[c][sel]] = np.ascontiguousarray(
            oc[:, sel].T).astype(np.float32)
    return full


# revision 27
# speedup vs baseline: 1.2471x; 1.0064x over previous
"""Trainium2 Bass kernel for nn_EdgeEncoder (moe_routing).

Strategy
--------
Each of E edges is routed to 1 of 9 expert MLPs (4 -> 256 -> 256), then
  out = relu(concat([type_embed[tid], source_embed[sid], pv]) @ Wf + bf).

Host (numpy, cheap O(E) work):
  * scale/mask params; group edges by expert (base type), and within an
    expert by (type_id, source_id) pair; DEAL each pair's edges round-robin
    across the 8 cores so every core sees identical pair-run boundaries
    (ceil(n_pair/8) columns each) -> one compiled program serves all cores,
  * algebraic fusions so the device does minimal work:
      - x gets a ones-row so b1 rides inside the layer-1 matmul,
      - V[t] = W2[t] @ Wf_pv (f64 host precompute) fuses layer 2 with the
        final projection: pv @ Wf_pv == h @ V[t] + const,
      - the whole "embedding" term  C[tid,sid] = type_embed[tid] @ Wf_t
        + source_embed[sid] @ Wf_s + b2[t] @ Wf_pv + bf  is constant per
        (tid,sid) pair.  Because edges are sorted by pair, C is piecewise
        constant along the edge axis and rides FOR FREE as the per-partition
        bias operand of the PSUM->SBUF relu evacuation ops.  This deletes
        the K=20 one-hot matmuls entirely (8 -> 6 PE passes per block).

Device per 512-edge block (edges pre-grouped, transposed):
  hT  = relu(W1e[t].T @ xT1)           2 matmuls fp32r K=5(pad 128) N=512
  outT[g] += V[t]-chunk(h).T @ hT[h]   4 matmuls bf16  K=128        N=512
  relu(outT + C[pair]) PSUM -> bf16 SBUF (bias-relu ops split between the
  Scalar and Vector engines, one slice per pair-run), DMA outT tiles to
  DRAM [D, L] bf16; host un-permutes and upcasts.

All matmuls are full 128-row (inputs zero-padded): partial-K matmuls run
in PE tile mode and starve the HAM activity monitor, dropping the PE to
1.2 GHz.  A short bf16 warm-up burst raises the HAM clock gate at kernel
start while the first DMAs are in flight.
"""

import math
import os

import ml_dtypes
import numpy as np

import concourse.bacc as bacc
import concourse.bass as bass
import concourse.mybir as mybir
import concourse.tile as tile
from concourse.bass_utils import run_bass_kernel_spmd

# ---- static module configuration (mirrors the torch source) ----
T = 9            # base types ("experts")
P_MAX = 4
D = 256
N_TYPES = 14
N_SRC = 5
NCORES = 8
BLOCK = 512      # edges per device block (one PSUM bank of fp32)
GRP = 8          # expert segment granularity (PSUM 8-byte-line alignment)
PDIM = 128       # PE partition dim

BASE_MAP = np.array([0, 0, 0, 1, 1, 1, 2, 2, 3, 4, 5, 6, 7, 8], dtype=np.int32)
PCOUNT = np.array([2, 2, 1, 1, 1, 1, 3, 2, 4], dtype=np.int32)
SCALES = np.ones((T, P_MAX), dtype=np.float32)
SCALES[0, :2] = [1.0, 1e-06]      # nmos  m, w
SCALES[1, :2] = [1.0, 1e-06]      # pmos  m, w
SCALES[2, 0] = 1.0                # balun rout
SCALES[3, 0] = 1000.0             # resistor r
SCALES[4, 0] = 1e-12              # capacitor c
SCALES[5, 0] = 1e-09              # inductor l
SCALES[6, :3] = [1.0, 1.0, 1.0]   # vsource dc, mag, phase
SCALES[7, :2] = [0.001, 0.001]    # isource dc, mag
SCALES[8, :4] = [1.0, 1.0, 1e9, 1.0]  # port dbm, dc, freq, num

KX = 5                            # x rows: xT(4) + ones
_F32 = mybir.dt.float32
_F32R = mybir.dt.float32r
_BF16 = mybir.dt.bfloat16
_WARM_BURST = int(os.environ.get("EDGEENC_WARM_BURST", "14"))

_PROGRAM_CACHE: dict = {}
LAST_RESULT = None  # BassKernelResults of the most recent run (for test harness)


def _layout(type_ids, source_ids):
    """Sort edges by (expert, type, source); deal each (t,tid,sid) group
    round-robin over cores.  Returns:
      ORD    [NCORES, L] global edge index per slot (-1 = pad)
      m_t    [T] per-core expert segment sizes (multiples of GRP)
      ranges list of (c0, c1, pair_col) bias runs, identical on all cores
             (pair_col 0 = pad columns, skipped by the device entirely)
      pairs  list of (t, tid, sid) in pair_col order (col 0 = zero bias)
      L      padded per-core column count (multiple of BLOCK)
    """
    base_ids = BASE_MAP[type_ids]
    m_t = np.zeros(T, dtype=np.int64)
    cols = []          # per expert: list of (k_p, pair_id, idx array)
    pairs = [None]     # pair_col 0 reserved for zero bias (pad columns)
    ranges = []
    ORD_parts = []
    off = 0
    for t in range(T):
        sel = np.nonzero(base_ids == t)[0]
        if sel.shape[0] == 0:
            continue
        key = type_ids[sel].astype(np.int64) * N_SRC + source_ids[sel]
        order = np.argsort(key, kind="stable")
        sel = sel[order]
        key = key[order]
        uk, counts = np.unique(key, return_counts=True)
        seg = 0
        pos = 0
        ord_t = []
        for u, n in zip(uk, counts):
            k_p = math.ceil(n / NCORES)
            pcol = len(pairs)
            pairs.append((t, int(u) // N_SRC, int(u) % N_SRC))
            idx = np.full(NCORES * k_p, -1, dtype=np.int64)
            idx[:n] = sel[pos : pos + n]
            pos += n
            # slot j of core c <- idx[j*NCORES + c]
            ord_t.append(idx.reshape(k_p, NCORES).T)     # [NCORES, k_p]
            ranges.append((off + seg, off + seg + k_p, pcol))
            seg += k_p
        m = math.ceil(seg / GRP) * GRP
        if m > seg:
            ranges.append((off + seg, off + m, 0))
            ord_t.append(np.full((NCORES, m - seg), -1, dtype=np.int64))
        m_t[t] = m
        ORD_parts.append(np.concatenate(ord_t, axis=1))
        off += m
    L0 = off
    L = math.ceil(L0 / BLOCK) * BLOCK
    if L > L0:
        ranges.append((L0, L, 0))
        ORD_parts.append(np.full((NCORES, L - L0), -1, dtype=np.int64))
        m_t[np.nonzero(m_t)[0][-1]] += L - L0
    ORD = np.concatenate(ORD_parts, axis=1)
    return ORD, m_t, ranges, pairs, L


def _host_inputs(type_ids, source_ids, params, ORD):
    """INX[c] = [5, L] bf16: xT (scaled/masked) + ones row."""
    base_ids = BASE_MAP[type_ids]
    scales = SCALES[base_ids]                                  # [E,4]
    validp = np.arange(P_MAX)[None, :] < PCOUNT[base_ids][:, None]
    x = np.where(validp, params.astype(np.float32) / scales, 0.0).astype(np.float32)

    L = ORD.shape[1]
    INX = np.zeros((NCORES, KX, L), dtype=np.float32)
    valid = ORD >= 0
    ids = ORD[valid]
    tmp = np.zeros((NCORES, L, P_MAX), dtype=np.float32)
    tmp[valid] = x[ids]
    INX[:, 0:P_MAX, :] = tmp.transpose(0, 2, 1)
    INX[:, P_MAX, :] = valid
    return INX.astype(ml_dtypes.bfloat16)


def _host_weights(type_embed, source_embed, W1, b1, W2, b2, Wf, bf, pairs):
    f = np.float32
    W1 = W1.astype(f); b1 = b1.astype(f); W2 = W2.astype(np.float64)
    b2 = b2.astype(np.float64); Wf = Wf.astype(np.float64); bf = bf.astype(np.float64)
    type_embed = type_embed.astype(np.float64)
    source_embed = source_embed.astype(np.float64)

    # layer1 lhsT blocks: [128, 9*256]; block t at cols [t*256,(t+1)*256),
    # zero-padded to 128 rows host-side (device DMAs it straight in)
    W1e = np.concatenate([W1, b1.astype(f)[:, None, :]], axis=1)   # [9,5,256]
    W1E = np.zeros((128, T * D), dtype=ml_dtypes.bfloat16)
    W1E[0:KX, :] = np.ascontiguousarray(
        W1e.transpose(1, 0, 2).reshape(KX, T * D)).astype(ml_dtypes.bfloat16)

    Wft, Wfs, Wfp = Wf[:D], Wf[D : 2 * D], Wf[2 * D :]

    # V[t] = W2[t] @ Wf_pv (f64), fusing layer 2 with the final projection.
    # bf16 lhsT blocks: [128, 18*256]; block (t,h) = V[t][h*128:(h+1)*128,:]
    V = (W2 @ Wfp).astype(f)                                        # [9,256,256]
    VR = np.ascontiguousarray(
        V.reshape(T, 2, 128, D).transpose(2, 0, 1, 3).reshape(128, T * 2 * D)
    ).astype(ml_dtypes.bfloat16)

    # bias table: C[pair=(t,tid,sid)] = te[tid]@Wft + se[sid]@Wfs + b2[t]@Wfp
    # + bf, stored transposed [128, 2*NPC] (half g at cols g*NPC + pcol)
    NPC = len(pairs)
    CG = np.zeros((128, 2 * NPC), dtype=f)
    for pcol in range(1, NPC):
        t, tid, sid = pairs[pcol]
        c = (type_embed[tid] @ Wft + source_embed[sid] @ Wfs
             + b2[t] @ Wfp + bf).astype(f)                          # [256]
        CG[:, pcol] = c[0:128]
        CG[:, NPC + pcol] = c[128:256]
    return W1E, VR, CG


def _build_program(m_t: tuple, L: int, ranges: tuple, NPC: int):
    """One compiled SPMD program for the given segment/bias-run layout."""
    key = (m_t, L, ranges, NPC, _WARM_BURST)
    if key in _PROGRAM_CACHE:
        return _PROGRAM_CACHE[key]

    group_expert = np.repeat(np.arange(T), (np.asarray(m_t) // GRP))
    NB = L // BLOCK
    GP = BLOCK // GRP  # groups per block = 4

    nc = bacc.Bacc("TRN2", target_bir_lowering=False, debug=False,
                   num_devices=NCORES)
    inx_d = nc.dram_tensor("inx", [KX, L], _BF16, kind="ExternalInput")
    w1e_d = nc.dram_tensor("w1e", [128, T * D], _BF16, kind="ExternalInput")
    vr_d = nc.dram_tensor("vr", [128, T * 2 * D], _BF16, kind="ExternalInput")
    cg_d = nc.dram_tensor("cg", [128, 2 * NPC], _F32, kind="ExternalInput")
    out_d = nc.dram_tensor("out", [D, L], _BF16, kind="ExternalOutput")

    RELU = mybir.ActivationFunctionType.Relu
    ADD = mybir.AluOpType.add
    MAX = mybir.AluOpType.max

    with tile.TileContext(nc) as tc:
        with (
            tc.tile_pool(name="wts", bufs=1) as wts,
            tc.tile_pool(name="inp", bufs=1) as inp,
            tc.tile_pool(name="hsb", bufs=6) as hsbp,
            tc.tile_pool(name="osb", bufs=6) as osbp,
            tc.tile_pool(name="hps", bufs=4, space=bass.MemorySpace.PSUM) as hps,
            tc.tile_pool(name="ops", bufs=4, space=bass.MemorySpace.PSUM) as ops,
        ):
            # HAM warm-up burst first: two tiny memsets at the head of the
            # vector queue, then bf16 matmuls into a scratch PSUM bank that
            # un-throttle the PE clock while the input DMAs are in flight
            if _WARM_BURST:
                wmw = wts.tile([128, 128], _BF16)
                wma = wts.tile([128, 256], _BF16)
                nc.vector.memset(wmw[:], 0.0)
                nc.vector.memset(wma[:], 0.0)
                wmp = hps.tile([PDIM, BLOCK], _F32, name="warmps", tag="hts")
                for i in range(_WARM_BURST):
                    nc.tensor.matmul(wmp[:, 0:256], wmw[:], wma[:], start=True,
                                     stop=True)

            w1e = wts.tile([128, T * D], _BF16)
            vr = wts.tile([128, T * 2 * D], _BF16)
            cg = wts.tile([128, 2 * NPC], _F32)
            # w1e arrives host-side zero-padded to all 128 rows (590KB bf16)
            # so no device memset sits ahead of the input DMAs
            nc.gpsimd.dma_start(w1e[:], w1e_d.ap())
            nc.gpsimd.dma_start(cg[:], cg_d.ap())
            # V (1.13MB bf16): expert 0's chunk first (the first ~7 blocks
            # are all expert 0), remainder split across the sync + scalar
            # queues, so the first V matmuls aren't stalled on the full load
            HALFV = T * D  # = half of the vr columns
            nc.sync.dma_start(vr[:, 0 : 2 * D], vr_d.ap()[:, 0 : 2 * D])
            nc.sync.dma_start(vr[:, 2 * D : HALFV], vr_d.ap()[:, 2 * D : HALFV])
            nc.scalar.dma_start(vr[:, HALFV:], vr_d.ap()[:, HALFV:])

            # persistent input buffers; zero-pad rows [KX:128) once so the
            # layer-1 matmul can run full-row (pads split over two engines)
            NIB = min(8, NB)
            PREF = min(4, NIB)
            xts = [inp.tile([128, BLOCK], _BF16, name=f"xtile{j}", tag=f"xtile{j}")
                   for j in range(NIB)]
            for b in range(min(PREF, NB)):
                nc.vector.memset(xts[b][:], 0.0)
                nc.gpsimd.dma_start(
                    xts[b][0:KX, :], inx_d.ap()[:, b * BLOCK : (b + 1) * BLOCK])
            for j in range(PREF, NIB):
                nc.vector.memset(xts[j][:], 0.0)

            # bias runs clipped per block; pad runs (pcol 0) are skipped and
            # also clipped out of the matmuls/evacuation/stores below
            block_ranges = [[] for _ in range(NB)]
            for (c0, c1, pcol) in ranges:
                if pcol == 0:
                    continue
                b0, b1 = c0 // BLOCK, (c1 - 1) // BLOCK
                for b in range(b0, b1 + 1):
                    r0 = max(c0, b * BLOCK) - b * BLOCK
                    r1 = min(c1, (b + 1) * BLOCK) - b * BLOCK
                    block_ranges[b].append((r0, r1, pcol))
            # pad groups: GRP-col groups lying fully inside a pad range
            is_pad = np.zeros(L // GRP, dtype=bool)
            for (c0, c1, pcol) in ranges:
                if pcol == 0:
                    g0p = (c0 + GRP - 1) // GRP
                    for g in range(g0p, c1 // GRP):
                        is_pad[g] = True

            evac_rr = 0  # round-robin bias-relu ops across Scalar/Vector
            for b in range(NB):
                g0 = b * GP
                experts = [int(group_expert[g0 + i]) for i in range(GP)]
                runs = []
                for i, t in enumerate(experts):
                    if runs and runs[-1][0] == t:
                        runs[-1] = (t, runs[-1][1], (i + 1) * GRP)
                    else:
                        runs.append((t, i * GRP, (i + 1) * GRP))
                # clip trailing pad groups out of each run (drop empty runs)
                cruns = []
                for (t, c0, c1) in runs:
                    while c1 > c0 and is_pad[g0 + c1 // GRP - 1]:
                        c1 -= GRP
                    if c1 > c0:
                        cruns.append((t, c0, c1))
                runs = cruns
                if not runs:
                    continue
                vend = max(c1 for (_, _, c1) in runs)

                xt = xts[b % NIB]
                bp = b + PREF
                if bp < NB:
                    xtp = xts[bp % NIB]
                    nc.gpsimd.dma_start(
                        xtp[0:KX, :],
                        inx_d.ap()[:, bp * BLOCK : (bp + 1) * BLOCK])

                # ---- layer 1: hT[h] = relu(W1e[t].T @ xT1), fp32r ----
                hts = [hps.tile([PDIM, BLOCK], _F32, name=f"hts{b}_{j}", tag="hts")
                       for j in range(2)]
                for (t, c0, c1) in runs:
                    for h in range(2):
                        nc.tensor.matmul(
                            hts[h][:, c0:c1],
                            w1e[:, t * D + h * PDIM : t * D + (h + 1) * PDIM],
                            xt[:, c0:c1],
                            start=True, stop=True,
                        )
                hsb = [hsbp.tile([PDIM, BLOCK], _BF16, name=f"hsb{b}_{j}", tag="hsb")
                       for j in range(2)]
                nc.scalar.activation(hsb[0][:, 0:vend], hts[0][:, 0:vend], RELU)
                nc.vector.tensor_scalar_max(hsb[1][:, 0:vend], hts[1][:, 0:vend],
                                            0.0)

                # ---- V stage: outT[g] += V[t]-chunk(h).T @ hT[h], bf16 ----
                ots = [ops.tile([PDIM, BLOCK], _F32, name=f"ots{b}_{j}", tag="ots")
                       for j in range(2)]
                for (t, c0, c1) in runs:
                    for h in range(2):
                        for g in range(2):
                            nc.tensor.matmul(
                                ots[g][:, c0:c1],
                                vr[:, (t * 2 + h) * D + g * PDIM
                                   : (t * 2 + h) * D + (g + 1) * PDIM],
                                hsb[h][:, c0:c1],
                                start=(h == 0), stop=(h == 1),
                            )

                # ---- bias-relu evacuation: out = relu(psum + C[pair]) ----
                osb = [osbp.tile([PDIM, BLOCK], _BF16, name=f"osb{b}_{j}", tag="osb")
                       for j in range(2)]
                for (r0, r1, pcol) in block_ranges[b]:
                    for g in range(2):
                        bias_ap = cg[:, g * NPC + pcol : g * NPC + pcol + 1]
                        if evac_rr % 2 == 0:
                            nc.scalar.activation(
                                osb[g][:, r0:r1], ots[g][:, r0:r1], RELU,
                                bias=bias_ap)
                        else:
                            nc.vector.tensor_scalar(
                                osb[g][:, r0:r1], ots[g][:, r0:r1],
                                bias_ap, 0.0, op0=ADD, op1=MAX)
                        evac_rr += 1
                for g in range(2):
                    nc.sync.dma_start(
                        out_d.ap()[g * PDIM : (g + 1) * PDIM,
                                   b * BLOCK : b * BLOCK + vend],
                        osb[g][:, 0:vend],
                    )

    nc.compile()
    _PROGRAM_CACHE[key] = nc
    return nc


def kernel(type_ids, source_ids, params, type_embed, source_embed,
           W1, b1, W2, b2, Wf, bf):
    global LAST_RESULT
    type_ids = np.asarray(type_ids, dtype=np.int32)
    source_ids = np.asarray(source_ids, dtype=np.int32)
    params = np.asarray(params, dtype=np.float32)
    E = type_ids.shape[0]

    ORD, m_t, ranges, pairs, L = _layout(type_ids, source_ids)
    INX = _host_inputs(type_ids, source_ids, params, ORD)
    W1E, VR, CG = _host_weights(
        np.asarray(type_embed), np.asarray(source_embed),
        np.asarray(W1), np.asarray(b1), np.asarray(W2), np.asarray(b2),
        np.asarray(Wf), np.asarray(bf), pairs)

    nc = _build_program(tuple(int(v) for v in m_t), L,
                        tuple(ranges), len(pairs))

    in_maps = [{"inx": np.ascontiguousarray(INX[c]), "w1e": W1E, "vr": VR,
                "cg": CG} for c in range(NCORES)]

    trace = bool(int(os.environ.get("EDGEENC_TRACE", "0")))
    res = run_bass_kernel_spmd(nc, in_maps, core_ids=list(range(NCORES)),
                               trace=trace)
    LAST_RESULT = res

    full = np.zeros((E, D), dtype=np.float32)
    for c in range(NCORES):
        sel = ORD[c] >= 0
        oc = res.results[c]["out"]                     # [D, L] bf16
        full[ORD[c][sel]] = np.ascontiguousarray(
            oc[:, sel].T).astype(np.float32)
    return full
